# revision 24
# baseline (speedup 1.0000x reference)
"""Trainium2 Bass kernel for a 5-layer MPNN (gnn_message_passing).

Model (per layer):  h = x @ Wl + bl
                    m = segment_sum(h[src], dst)          # 1.6M edges
                    x = leaky_relu(cat(h, m) @ Wu + bu, 0.1)
Then global mean-pool over 256 graphs, a [128,2] linear, log_softmax.

Distribution: nodes are partitioned across 8 NeuronCores at graph
boundaries (data parallel over graphs); edges are owned by the core that
owns their destination node so the scatter-add is core-local.  Each layer
the (bf16) node features h are AllGather'd so every core can gather its
edges' source rows with `dma_gather`.  The segment-sum is performed on the
TensorEngine: for each 128-edge tile a host-precomputed one-hot matrix
(edge -> destination slot) is used as the matmul moving operand so PSUM
accumulates messages per destination window.

The per-core *data* differs but the instruction stream is identical
(single SPMD NEFF): the host packs each core's destination nodes into
N_WIN windows of 128 slots such that every (window, src-chunk) group has
at most WCAP=512 edges; the streams are padded to exactly WCAP, giving a
static schedule shared by all cores.
"""

import numpy as np
import ml_dtypes

# ---------------------------------------------------------------- constants
N_NODES = 100000
N_EDGES = 1600000
IN_CH = 128
HID = 128
N_GRAPHS = 256
NEG_SLOPE = 0.1

NCORES = 8
N_WIN = 104              # destination windows per core (128 slots each)
N_SLOTS = N_WIN * 128    # 13312 node slots per core
NCHUNK = 4               # src chunks (int16 index range for dma_gather)
CHUNK_ROWS = NCORES * N_SLOTS // NCHUNK  # 26624 rows of h_full per chunk
WCAP = 512               # edge slots per (window, chunk)
TILES_WC = WCAP // 128   # 4
BATCH_W = 4              # windows per gather batch == 512-col z chunk
NBATCH = N_WIN // BATCH_W            # 26
IDX_PER_GATHER = BATCH_W * WCAP      # 2048
STREAM = N_WIN * NCHUNK * WCAP       # 212992 edge slots per core
N_TILES = STREAM // 128              # 1664
G_SLOTS = 64             # graph slots per core
H_ROWS = NCORES * N_SLOTS            # 106496 rows in h_full

_CACHE = {}
# debug knobs (used by dev harness only; defaults = full kernel)
_DEBUG = {"layers": 5, "scatter": True, "collective": True, "use_dma_gather": True,
          "scatter_mm": True, "nbatch": None, "repeat": 1}


# ------------------------------------------------------------- host planning
def _partition_graphs(batch):
    counts = np.bincount(batch, minlength=N_GRAPHS)
    cum = np.concatenate([[0], np.cumsum(counts)])
    cuts = [0]
    for c in range(1, NCORES):
        target = N_NODES * c // NCORES
        g = int(np.searchsorted(cum, target))
        if g > 0 and target - cum[g - 1] < cum[g] - target:
            g -= 1
        cuts.append(g)
    cuts.append(N_GRAPHS)
    parts = []
    for c in range(NCORES):
        g0, g1 = cuts[c], cuts[c + 1]
        parts.append((g0, g1, int(cum[g0]), int(cum[g1])))
        assert cum[g1] - cum[g0] <= N_SLOTS
        assert g1 - g0 <= G_SLOTS
    return parts, counts


def _pack_windows(deg):
    """deg [n_loc, NCHUNK] -> window id per node s.t. per (window, chunk)
    load <= WCAP and window node count <= 128."""
    n_loc = deg.shape[0]
    loads = np.zeros((N_WIN, NCHUNK), np.int64)
    cnt = np.zeros(N_WIN, np.int64)
    win = np.zeros(n_loc, np.int64)
    order = np.argsort(-deg.sum(axis=1), kind="stable")
    for i in order:
        d = deg[i]
        new = loads + d
        feas = (new <= WCAP).all(axis=1) & (cnt < 128)
        if not feas.any():
            raise RuntimeError("window packing infeasible")
        score = new.max(axis=1) * 1000 + cnt * 40
        score[~feas] = 1 << 60
        w = int(np.argmin(score))
        win[i] = w
        loads[w] += d
        cnt[w] += 1
    return win


def _plan(x, src, dst, batch):
    parts, gcounts = _partition_graphs(batch)
    node_core = np.zeros(N_NODES, np.int64)
    for c, (g0, g1, n0, n1) in enumerate(parts):
        node_core[n0:n1] = c
    chunk_of_src = node_core[src] // 2
    dst_core = node_core[dst]

    # window packing per core, then global slot map
    wins, edges = [], []
    for c in range(NCORES):
        g0, g1, n0, n1 = parts[c]
        emask = dst_core == c
        e_src, e_dst, e_ch = src[emask], dst[emask] - n0, chunk_of_src[emask]
        deg = np.zeros((n1 - n0, NCHUNK), np.int64)
        np.add.at(deg, (e_dst, e_ch), 1)
        wins.append(_pack_windows(deg))
        edges.append((e_src, e_dst, e_ch))

    # slot of each node within its core: window w, position = order inside w
    g_slot = np.zeros(N_NODES, np.int64)  # slot within owner core
    slot_node = []                        # per core: local node at each slot (-1 pad)
    for c in range(NCORES):
        g0, g1, n0, n1 = parts[c]
        win = wins[c]
        n_loc = n1 - n0
        order = np.lexsort((np.arange(n_loc), win))  # stable by window
        sn = np.full(N_SLOTS, -1, np.int64)
        pos_in_win = np.zeros(n_loc, np.int64)
        wsorted = win[order]
        # position within window = running index per window
        boundaries = np.concatenate([[0], np.cumsum(np.bincount(wsorted, minlength=N_WIN))])
        for w in range(N_WIN):
            seg = order[boundaries[w]:boundaries[w + 1]]
            sn[w * 128:w * 128 + len(seg)] = seg
            pos_in_win[seg] = np.arange(len(seg))
        g_slot[n0:n1] = win * 128 + pos_in_win
        slot_node.append(sn)

    return parts, node_core, g_slot, slot_node, wins, edges, gcounts


def _build_core_inputs(c, x, batch, parts, node_core, g_slot, slot_node, wins,
                       edges, gcounts):
    g0, g1, n0, n1 = parts[c]
    n_loc = n1 - n0
    sn = slot_node[c]
    e_src, e_dst, e_ch = edges[c]

    # xT [128, N_SLOTS] f32 (feature-major, permuted node slots, pad 0)
    xT = np.zeros((128, N_SLOTS), np.float32)
    valid = sn >= 0
    xT[:, valid] = x[n0 + sn[valid], :].T

    # edge stream: slot position of every edge
    w_of_e = wins[c][e_dst]
    # stream layout: gather instruction s = b*NCHUNK+c covers slots
    # [s*2048, ..); window wi (within batch) at sub-range wi*512.
    b_of_e = w_of_e // BATCH_W
    wi_of_e = w_of_e % BATCH_W
    group_key = (b_of_e * NCHUNK + e_ch) * BATCH_W + wi_of_e
    order = np.argsort(group_key, kind="stable")
    gk_sorted = group_key[order]
    counts = np.bincount(gk_sorted, minlength=N_WIN * NCHUNK)
    cum = np.concatenate([[0], np.cumsum(counts)])
    pos_sorted = np.arange(len(order)) - cum[gk_sorted]  # rank within group
    assert (pos_sorted < WCAP).all()
    gbase = (b_of_e * NCHUNK + e_ch) * IDX_PER_GATHER + wi_of_e * WCAP
    slot_of_e = np.zeros(len(order), np.int64)
    slot_of_e[order] = gbase[order] + pos_sorted

    # gather index stream (int16, pad -> row 0 of chunk)
    idx_stream = np.zeros(STREAM, np.int16)
    src_row_rel = (node_core[e_src] % 2) * N_SLOTS + g_slot[e_src]
    assert (src_row_rel < CHUNK_ROWS).all()
    idx_stream[slot_of_e] = src_row_rel.astype(np.int16)
    # wrapped [128, STREAM/16]
    w16 = idx_stream.reshape(-1, 16).T  # [16, STREAM/16]
    idxw = np.tile(w16, (8, 1)).copy()

    # one-hot stream [128, N_TILES*128] bf16
    oh = np.zeros((STREAM, 128), ml_dtypes.bfloat16)
    oh[slot_of_e, g_slot[n0 + e_dst] % 128] = 1
    onehot = np.ascontiguousarray(
        oh.reshape(N_TILES, 128, 128).transpose(1, 0, 2).reshape(128, N_TILES * 128))

    # pool matrix [128, N_WIN * G_SLOTS] f32: P[p, t*G_SLOTS+g] for slot t*128+p
    poolm = np.zeros((128, N_WIN * G_SLOTS), np.float32)
    gl = batch[n0 + sn[valid]] - g0            # local graph id per valid slot
    slots = np.nonzero(valid)[0]
    t, p = slots // 128, slots % 128
    poolm[p, t * G_SLOTS + gl] = 1.0 / gcounts[g0 + gl]
    return dict(xT=xT, idxw=idxw, onehot=onehot, poolm=poolm)


# ------------------------------------------------------------- bass program
def _build_nc():
    import concourse.bass as bass
    import concourse.mybir as mybir
    import concourse.tile as tile
    from concourse import bacc
    from concourse.masks import make_identity
    from contextlib import ExitStack

    dt = mybir.dt
    Alu = mybir.AluOpType
    Act = mybir.ActivationFunctionType

    nc = bacc.Bacc("TRN2", target_bir_lowering=False, debug=False,
                   num_devices=NCORES)

    xT_d = nc.dram_tensor("xT", [128, N_SLOTS], dt.float32, kind="ExternalInput")
    idxw_d = nc.dram_tensor("idxw", [128, STREAM // 16], dt.int16, kind="ExternalInput")
    oh_d = nc.dram_tensor("onehot", [128, N_TILES * 128], dt.bfloat16, kind="ExternalInput")
    pool_d = nc.dram_tensor("poolm", [128, N_WIN * G_SLOTS], dt.float32, kind="ExternalInput")
    wl_d = nc.dram_tensor("wl", [128, 5 * 128], dt.float32, kind="ExternalInput")
    wuh_d = nc.dram_tensor("wuh", [128, 5 * 128], dt.float32, kind="ExternalInput")
    wum_d = nc.dram_tensor("wum", [128, 5 * 128], dt.float32, kind="ExternalInput")
    bl_d = nc.dram_tensor("bl", [128, 5], dt.float32, kind="ExternalInput")
    bu_d = nc.dram_tensor("bu", [128, 5], dt.float32, kind="ExternalInput")
    wout_d = nc.dram_tensor("wout", [128, 2], dt.float32, kind="ExternalInput")
    bout_d = nc.dram_tensor("bout", [2, 1], dt.float32, kind="ExternalInput")
    out_d = nc.dram_tensor("out_logits", [G_SLOTS, 2], dt.float32, kind="ExternalOutput")

    with tile.TileContext(nc) as tc, ExitStack() as ctx:
        consts = ctx.enter_context(tc.tile_pool(name="consts", bufs=1))
        acts = ctx.enter_context(tc.tile_pool(name="acts", bufs=1))
        stage = ctx.enter_context(tc.tile_pool(name="stage", bufs=3))
        gpool = ctx.enter_context(tc.tile_pool(name="gpool", bufs=3))
        ohpool = ctx.enter_context(tc.tile_pool(name="ohpool", bufs=3))
        ixpool = ctx.enter_context(tc.tile_pool(name="ixpool", bufs=3))
        mpool = ctx.enter_context(tc.tile_pool(name="mpool", bufs=5))
        ztmp = ctx.enter_context(tc.tile_pool(name="ztmp", bufs=3))
        small = ctx.enter_context(tc.tile_pool(name="small", bufs=2))
        spsum = ctx.enter_context(tc.tile_pool(name="spsum", bufs=4, space="PSUM"))
        hzpsum = ctx.enter_context(tc.tile_pool(name="hzpsum", bufs=2, space="PSUM"))
        tpsum = ctx.enter_context(tc.tile_pool(name="tpsum", bufs=2, space="PSUM"))
        dram = ctx.enter_context(tc.tile_pool(name="dram", bufs=1, space="DRAM"))

        # constants
        wl_s = consts.tile([128, 5 * 128], dt.float32, tag="wl")
        wuh_s = consts.tile([128, 5 * 128], dt.float32, tag="wuh")
        wum_s = consts.tile([128, 5 * 128], dt.float32, tag="wum")
        bl_s = consts.tile([128, 5], dt.float32, tag="bl")
        bu_s = consts.tile([128, 5], dt.float32, tag="bu")
        wout_s = consts.tile([128, 2], dt.float32, tag="wout")
        bout_s = consts.tile([2, 1], dt.float32, tag="bout")
        ident = consts.tile([128, 128], dt.float32, tag="ident")
        for s, d in [(wl_s, wl_d), (wuh_s, wuh_d), (wum_s, wum_d),
                     (bl_s, bl_d), (bu_s, bu_d), (wout_s, wout_d), (bout_s, bout_d)]:
            nc.sync.dma_start(out=s[:], in_=d[:])
        make_identity(nc, ident[:])

        xT = acts.tile([128, N_SLOTS], dt.float32, tag="xT")
        hT = acts.tile([128, N_SLOTS], dt.float32, tag="hT")
        nc.sync.dma_start(out=xT[:], in_=xT_d[:])

        h_slice = dram.tile([N_SLOTS, 128], dt.bfloat16, tag="h_slice")

        NZ = N_SLOTS // 512  # 26 z/h chunks

        for rep in range(_DEBUG["repeat"]):
         for layer0 in range(_DEBUG["layers"]):
            layer = layer0 + 5 * rep  # unique names per repetition
            wl_ap = wl_s[:, layer0 * 128:(layer0 + 1) * 128]
            wuh_ap = wuh_s[:, layer0 * 128:(layer0 + 1) * 128]
            wum_ap = wum_s[:, layer0 * 128:(layer0 + 1) * 128]
            bl_ap = bl_s[:, layer0:layer0 + 1]
            bu_ap = bu_s[:, layer0:layer0 + 1]

            # ---- h.T = Wl.T @ x.T + bl ; stage bf16 node-major slice to DRAM
            for k in range(NZ):
                ph = hzpsum.tile([128, 512], dt.float32, tag="hz")
                nc.tensor.matmul(out=ph[:], lhsT=wl_ap, rhs=xT[:, k * 512:(k + 1) * 512],
                                 start=True, stop=True, skip_group_check=True)
                nc.vector.tensor_scalar(out=hT[:, k * 512:(k + 1) * 512], in0=ph[:],
                                        scalar1=bl_ap, scalar2=None, op0=Alu.add)
                st = stage.tile([128, 4, 128], dt.bfloat16, tag="st")
                for j in range(4):
                    t = k * 4 + j
                    pt = tpsum.tile([128, 128], dt.float32, tag="tp")
                    nc.tensor.transpose(out=pt[:], in_=hT[:, t * 128:(t + 1) * 128],
                                        identity=ident[:])
                    nc.vector.tensor_copy(out=st[:, j, :], in_=pt[:])
                nc.sync.dma_start(
                    out=h_slice[k * 512:(k + 1) * 512, :].rearrange(
                        "(j p) f -> p j f", p=128),
                    in_=st[:])

            # ---- AllGather h (bf16)
            h_full = dram.tile([H_ROWS, 128], dt.bfloat16, tag=f"h_full_{layer}",
                               addr_space="Shared" if _DEBUG["collective"] else "Local",
                               name=f"h_full_{layer}")
            if _DEBUG["collective"]:
                nc.gpsimd.collective_compute(
                    "AllGather", mybir.AluOpType.bypass,
                    replica_groups=[list(range(NCORES))],
                    ins=[h_slice.opt()], outs=[h_full.opt()],
                )
            else:
                nc.sync.dma_start(out=h_full[:N_SLOTS, :], in_=h_slice[:])
            # dma_gather ignores in_ap offsets -> materialize the 4 chunks
            h_chunks = []
            for c in range(NCHUNK):
                hc = dram.tile([CHUNK_ROWS, 128], dt.bfloat16,
                               tag=f"hc_{c}", name=f"hc_{layer}_{c}")
                nc.sync.dma_start(out=hc[:],
                                  in_=h_full[c * CHUNK_ROWS:(c + 1) * CHUNK_ROWS, :])
                h_chunks.append(hc)

            # ---- gather + scatter + z, batch by batch
            for b in range(0 if not _DEBUG["scatter"] else
                           (_DEBUG["nbatch"] or NBATCH)):
                gt = []
                ot = []
                for c in range(NCHUNK):
                    s = b * NCHUNK + c
                    ix = ixpool.tile([128, 128], dt.int16, tag="ix")
                    nc.sync.dma_start(out=ix[:], in_=idxw_d[:, s * 128:(s + 1) * 128])
                    g = gpool.tile([128, 16, 128], dt.bfloat16, tag="g")
                    if _DEBUG["use_dma_gather"]:
                        nc.gpsimd.dma_gather(
                            out_ap=g[:],
                            in_ap=h_chunks[c][:],
                            idxs_ap=ix[:], num_idxs=IDX_PER_GATHER,
                            num_idxs_reg=IDX_PER_GATHER, elem_size=128,
                            single_packet=False)
                    else:
                        nc.sync.dma_start(
                            out=g[:],
                            in_=h_full[:IDX_PER_GATHER, :].rearrange(
                                "(a p) f -> p a f", p=128))
                    o = ohpool.tile([128, 16, 128], dt.bfloat16, tag="oh")
                    nc.sync.dma_start(
                        out=o[:],
                        in_=oh_d[:, s * 2048:(s + 1) * 2048].rearrange(
                            "p (a b) -> p a b", b=128))
                    gt.append(g)
                    ot.append(o)
                pw = [spsum.tile([128, 128], dt.float32, tag="pw",
                                 name=f"pw_{layer}_{b}_{wi}") for wi in range(BATCH_W)]
                if _DEBUG["scatter_mm"]:
                    for c in range(NCHUNK):
                        for wi in range(BATCH_W):
                            for t in range(TILES_WC):
                                j = wi * TILES_WC + t
                                nc.tensor.matmul(
                                    out=pw[wi][:], lhsT=gt[c][:, j, :], rhs=ot[c][:, j, :],
                                    start=(c == 0 and t == 0),
                                    stop=(c == NCHUNK - 1 and t == TILES_WC - 1),
                                    skip_group_check=True)
                else:
                    for wi in range(BATCH_W):
                        nc.tensor.matmul(
                            out=pw[wi][:], lhsT=gt[0][:, wi, :], rhs=ot[0][:, wi, :],
                            start=True, stop=True, skip_group_check=True)
                # z chunk for this batch: columns [b*512, (b+1)*512)
                mws = []
                for wi in range(BATCH_W):
                    mw = mpool.tile([128, 128], dt.float32, tag="mw")
                    nc.vector.tensor_copy(out=mw[:], in_=pw[wi][:])
                    mws.append(mw)
                pz = hzpsum.tile([128, 512], dt.float32, tag="hz")
                nc.tensor.matmul(out=pz[:], lhsT=wuh_ap,
                                 rhs=hT[:, b * 512:(b + 1) * 512],
                                 start=True, stop=False, skip_group_check=True)
                for wi in range(BATCH_W):
                    nc.tensor.matmul(out=pz[:, wi * 128:(wi + 1) * 128], lhsT=wum_ap,
                                     rhs=mws[wi][:],
                                     start=False, stop=(wi == BATCH_W - 1),
                                     skip_group_check=True)
                t1 = ztmp.tile([128, 512], dt.float32, tag="t1")
                t2 = ztmp.tile([128, 512], dt.float32, tag="t2")
                nc.scalar.activation(out=t1[:], in_=pz[:], func=Act.Identity,
                                     bias=bu_ap, scale=1.0)
                nc.scalar.mul(out=t2[:], in_=t1[:], mul=NEG_SLOPE)
                nc.vector.tensor_tensor(out=xT[:, b * 512:(b + 1) * 512],
                                        in0=t1[:], in1=t2[:], op=Alu.max)

        # ---- global mean pool + linear + log_softmax
        pp = hzpsum.tile([G_SLOTS, 128], dt.float32, tag="hz")
        for t in range(N_WIN):
            pt = tpsum.tile([128, 128], dt.float32, tag="tp")
            nc.tensor.transpose(out=pt[:], in_=xT[:, t * 128:(t + 1) * 128],
                                identity=ident[:])
            xn = small.tile([128, 128], dt.float32, tag="xn")
            nc.vector.tensor_copy(out=xn[:], in_=pt[:])
            pm = small.tile([128, G_SLOTS], dt.float32, tag="pm")
            nc.sync.dma_start(out=pm[:], in_=pool_d[:, t * G_SLOTS:(t + 1) * G_SLOTS])
            nc.tensor.matmul(out=pp[:], lhsT=pm[:], rhs=xn[:],
                             start=(t == 0), stop=(t == N_WIN - 1),
                             skip_group_check=True)
        pooled = small.tile([G_SLOTS, 128], dt.float32, tag="pooled")
        nc.vector.tensor_copy(out=pooled[:], in_=pp[:])
        # pooled.T [128, G]
        ppt = tpsum.tile([128, G_SLOTS], dt.float32, tag="tp")
        nc.tensor.transpose(out=ppt[:], in_=pooled[:], identity=ident[:G_SLOTS, :G_SLOTS])
        pooledT = small.tile([128, G_SLOTS], dt.float32, tag="pooledT")
        nc.vector.tensor_copy(out=pooledT[:], in_=ppt[:])
        # logits.T [2, G] = W_out.T @ pooled.T + b_out
        pl = tpsum.tile([2, G_SLOTS], dt.float32, tag="tp")
        nc.tensor.matmul(out=pl[:], lhsT=wout_s[:], rhs=pooledT[:],
                         start=True, stop=True, skip_group_check=True)
        ltT = small.tile([2, G_SLOTS], dt.float32, tag="ltT")
        nc.vector.tensor_scalar(out=ltT[:], in0=pl[:], scalar1=bout_s[:],
                                scalar2=None, op0=Alu.add)
        # transpose -> [G, 2]
        plg = tpsum.tile([G_SLOTS, 2], dt.float32, tag="tp")
        nc.tensor.transpose(out=plg[:], in_=ltT[:], identity=ident[:2, :2])
        lg = small.tile([G_SLOTS, 2], dt.float32, tag="lg")
        nc.vector.tensor_copy(out=lg[:], in_=plg[:])
        # log_softmax along free dim (2)
        mx = small.tile([G_SLOTS, 1], dt.float32, tag="mx")
        nc.vector.tensor_reduce(out=mx[:], in_=lg[:], axis=mybir.AxisListType.X,
                                op=Alu.max)
        zc = small.tile([G_SLOTS, 2], dt.float32, tag="zc")
        nc.vector.tensor_scalar(out=zc[:], in0=lg[:], scalar1=mx[:],
                                scalar2=None, op0=Alu.subtract)
        ex = small.tile([G_SLOTS, 2], dt.float32, tag="ex")
        nc.scalar.activation(out=ex[:], in_=zc[:], func=Act.Exp)
        sm = small.tile([G_SLOTS, 1], dt.float32, tag="sm")
        nc.vector.tensor_reduce(out=sm[:], in_=ex[:], axis=mybir.AxisListType.X,
                                op=Alu.add)
        ln = small.tile([G_SLOTS, 1], dt.float32, tag="ln")
        nc.scalar.activation(out=ln[:], in_=sm[:], func=Act.Ln)
        res = small.tile([G_SLOTS, 2], dt.float32, tag="res")
        nc.vector.tensor_scalar(out=res[:], in0=zc[:], scalar1=ln[:],
                                scalar2=None, op0=Alu.subtract)
        nc.sync.dma_start(out=out_d[:], in_=res[:])

    nc.compile()
    return nc


# ------------------------------------------------------------------- driver
def _prepare(x, edge_index, batch, params):
    x = np.asarray(x, np.float32)
    src = np.asarray(edge_index[0], np.int64)
    dst = np.asarray(edge_index[1], np.int64)
    batch = np.asarray(batch, np.int64)

    parts, node_core, g_slot, slot_node, wins, edges, gcounts = _plan(
        x, src, dst, batch)

    # weights (shared across cores)
    wl = np.concatenate([np.asarray(params[i]["Wl"], np.float32) for i in range(5)], axis=1)
    wuh = np.concatenate([np.asarray(params[i]["Wu"], np.float32)[:128] for i in range(5)], axis=1)
    wum = np.concatenate([np.asarray(params[i]["Wu"], np.float32)[128:] for i in range(5)], axis=1)
    bl = np.stack([np.asarray(params[i]["bl"], np.float32) for i in range(5)], axis=1)
    bu = np.stack([np.asarray(params[i]["bu"], np.float32) for i in range(5)], axis=1)
    wout = np.asarray(params[5]["W_out"], np.float32)
    bout = np.asarray(params[5]["b_out"], np.float32).reshape(2, 1)

    in_maps = []
    for c in range(NCORES):
        d = _build_core_inputs(c, x, batch, parts, node_core, g_slot, slot_node,
                               wins, edges, gcounts)
        d.update(wl=wl, wuh=wuh, wum=wum, bl=bl, bu=bu, wout=wout, bout=bout)
        in_maps.append(d)
    return in_maps, parts


def kernel(x, edge_index, batch, params):
    from concourse.bass_utils import run_bass_kernel_spmd

    in_maps, parts = _prepare(x, edge_index, batch, params)

    if "nc" not in _CACHE:
        _CACHE["nc"] = _build_nc()
    nc = _CACHE["nc"]

    res = run_bass_kernel_spmd(nc, in_maps, core_ids=list(range(NCORES)))
    out = np.zeros((N_GRAPHS, 2), np.float32)
    for c in range(NCORES):
        g0, g1, _, _ = parts[c]
        out[g0:g1] = res.results[c]["out_logits"][: g1 - g0]
    return out
